# revision 16
# baseline (speedup 1.0000x reference)
"""Trainium2 Bass kernel for the batched Kalman filter problem.

Problem: G=2048 groups, T=256 steps, M=8 obs dims, S=16 state dims.
Output: means [G,T,M], covs [G,T,M,M] (the per-step predicted measurement
distribution).

Structure exploited:
  * The covariance recursion P_t is data-independent and P0 is broadcast, so
    P_t / the Kalman gain / the output covariance ycov_t are identical for
    every group.  ycov [T,M,M] is computed on host (tiny, sequential 16x16
    math) and the device broadcasts it into the [G,T,M,M] output — that
    broadcast write (134 MB) is the memory-bound bulk of the work.
  * The mean recursion is affine with shared coefficient matrices:
        mean_{t+1} = mean_t @ A_t + obs_t @ B_t,   ymean_t = mean_t @ H^T
    Chunking t into blocks of C=16 turns this into a short sequential chain
    of dense matmuls over the whole group batch (device, TensorE).

Sharding: groups split across 8 cores (256 groups/core), pure data parallel.

Device per-core layout (all f32):
  obs_pack  [128, 16*256] in  : col block k = obsT rows [128k:128k+128)
                                (obsT[(t*M+m), g] = input[g, t, m])
  w_pack    [128, 16*128] in  : col block k = W_k   (lhsT, K=(j,m') x N=(i,m))
  u_pack    [16,  16*128] in  : col block k = U_k   (lhsT, K=s x N=(i,m))
  acar_pack [16,  16*16]  in  : col block k = Acar_k (lhsT)
  bcar_pack [128, 16*16]  in  : col block k = Bcar_k (lhsT)
  mean0_t   [16, 256]     in  : initial mean broadcast over groups (meanT)
  ycov_row  [1, 16384]    in  : ycovs [T*M*M] flattened
  ymeant_out [2048, 256]  out : ymeanT[(t*M+m), g]  (host transposes back)
  covs_out   [256, 16384] out : per-group covs rows (identical), g-major
"""

import numpy as np

G, T, M, S = 2048, 256, 8, 16
NCORES = 8
G_SH = G // NCORES            # 256 groups per core
C = 16                        # time chunk
NCH = T // C                  # 16 chunks
CM = C * M                    # 128 = rows/cols of W per chunk
P = 128
YCN = T * M * M               # 16384 floats of ycov
REP_TILES = 4                 # split replicated ycov into 4 [128, 4096] tiles
REP_W = YCN // REP_TILES      # 4096

_CACHE: dict = {}


def _install_ntff_hook():
    """Provide antenv.axon_hooks (absent in this image) so bass_utils can
    NTFF-profile under axon when trace=True.  No-op if already present."""
    import sys
    import types
    import contextlib

    try:
        import antenv.axon_hooks  # noqa: F401
        return
    except ImportError:
        pass

    mod = types.ModuleType("antenv.axon_hooks")
    state = {"hook": None}
    mod.set_axon_ntff_profile_hook = lambda h: state.__setitem__("hook", h)
    mod.get_axon_ntff_profile_hook = lambda: state["hook"]
    sys.modules["antenv.axon_hooks"] = mod
    try:
        import antenv
        antenv.axon_hooks = mod
    except ImportError:
        pass

    so_path = "/opt/axon/libaxon_pjrt.so"
    try:
        import ctypes
        lib = ctypes.CDLL(so_path)
        if not hasattr(lib, "axon_start_nrt_profile"):
            return
        lib.axon_start_nrt_profile.argtypes = [
            ctypes.POINTER(ctypes.c_int64), ctypes.c_size_t]
        lib.axon_start_nrt_profile.restype = ctypes.c_int64
        lib.axon_stop_nrt_profile.argtypes = [ctypes.c_char_p]
        lib.axon_stop_nrt_profile.restype = ctypes.c_int64

        @contextlib.contextmanager
        def _hook(output_dir, device_ids):
            import jax
            jax.devices()
            if device_ids:
                ids = (ctypes.c_int64 * len(device_ids))(*device_ids)
                rc = lib.axon_start_nrt_profile(ids, len(device_ids))
            else:
                rc = lib.axon_start_nrt_profile(None, 0)
            if rc != 0:
                raise RuntimeError(f"axon_start_nrt_profile rc={rc}")
            try:
                yield
            finally:
                n = lib.axon_stop_nrt_profile(str(output_dir).encode())
                import sys as _s
                print(f"ntff profile: {n} file(s) -> {output_dir}", file=_s.stderr)

        state["hook"] = _hook
    except OSError:
        pass


_install_ntff_hook()


# ----------------------------------------------------------------------------
# Host-side math: tiny data-independent Kalman recursions (float64).
# ----------------------------------------------------------------------------

def _compute_params(init_state_mean, init_log_diag, init_off_diag, F, H, L_Q, L_R):
    F = np.asarray(F, np.float64)
    H = np.asarray(H, np.float64)
    L_Q = np.asarray(L_Q, np.float64)
    L_R = np.asarray(L_R, np.float64)
    Q = L_Q @ L_Q.T + 1e-4 * np.eye(S)
    R = L_R @ L_R.T + 1e-4 * np.eye(M)
    L0 = np.zeros((S, S))
    r, c = np.tril_indices(S, -1)
    L0[r, c] = np.asarray(init_off_diag, np.float64)
    L0 += np.diag(np.exp(np.asarray(init_log_diag, np.float64)))
    Pm = L0 @ L0.T

    I_S = np.eye(S)
    ycovs = np.zeros((T, M, M))
    A = np.zeros((T - 1, S, S))
    B = np.zeros((T - 1, M, S))
    ycovs[0] = H @ Pm @ H.T + R
    for j in range(T - 1):
        HP = H @ Pm
        Smat = HP @ H.T + R
        Kt = np.linalg.solve(Smat, HP)          # [M,S] = K^T
        P_u = Pm - Kt.T @ HP
        A[j] = (I_S - H.T @ Kt) @ F.T
        B[j] = Kt @ F.T
        Pm = F @ P_u @ F.T + Q
        ycovs[j + 1] = H @ Pm @ H.T + R

    Ht = H.T

    def getA(j):
        return A[j] if j < T - 1 else I_S

    def getB(j):
        return B[j] if j < T - 1 else np.zeros((M, S))

    U = np.zeros((NCH, S, CM))
    W = np.zeros((NCH, CM, CM))
    Acar = np.zeros((NCH, S, S))
    Bcar = np.zeros((NCH, CM, S))
    for k in range(NCH):
        t0 = k * C
        pref = I_S.copy()
        for i in range(C):
            U[k, :, i * M:(i + 1) * M] = pref @ Ht
            pref = pref @ getA(t0 + i)
        Acar[k] = pref
        for j in range(C):
            Bj = getB(t0 + j)
            mid = I_S.copy()
            for i in range(j + 1, C):
                W[k, j * M:(j + 1) * M, i * M:(i + 1) * M] = Bj @ mid @ Ht
                mid = mid @ getA(t0 + i)
            Bcar[k, j * M:(j + 1) * M, :] = Bj @ mid

    # super-chunk (4 chunks = 64 steps) transition composites for the
    # two-level scan: mt_{4(J+1)} = AcarS_J^T mt_{4J} + sum_i BcarS_J[i]^T obs_{4J+i}
    NSUP = NCH // 4 - 1                          # 3 boundary hops
    AcarS = np.zeros((NSUP, S, S))
    BcarS = np.zeros((NSUP, 4, CM, S))           # per obs chunk 4J+i
    for J in range(NSUP):
        pr = I_S.copy()
        for i in range(3, -1, -1):               # suffix products of Acar
            BcarS[J, i] = Bcar[4 * J + i] @ pr
            pr = Acar[4 * J + i] @ pr
        AcarS[J] = pr

    def pack(x):  # [NCH, p, n] -> [p, NCH*n], col block k = x[k]
        return np.ascontiguousarray(
            x.transpose(1, 0, 2).reshape(x.shape[1], -1)).astype(np.float32)

    mean0 = np.asarray(init_state_mean, np.float64)
    # two packed param tensors (one DMA each) to keep total DMA count low —
    # Tile has only 8 DMA-completion sem lanes and lane reuse creates false
    # cross-DMA ordering when late DMAs precede early ones in lane order.
    pp128 = np.concatenate([
        pack(W),                                           # [:, 0:2048]
        pack(Bcar),                                        # [:, 2048:2304]
        np.ascontiguousarray(                              # [:, 2304:2496]
            BcarS.transpose(2, 0, 1, 3).reshape(CM, -1)).astype(np.float32),
    ], axis=1)
    pp16 = np.concatenate([
        pack(U),                                           # [:, 0:2048]
        pack(Acar),                                        # [:, 2048:2304]
        pack(AcarS),                                       # [:, 2304:2352]
        np.ascontiguousarray(                              # [:, 2352:2608]
            np.broadcast_to(mean0[:, None], (S, G_SH))).astype(np.float32),
    ], axis=1)
    return {
        "pp128": np.ascontiguousarray(pp128),    # [128, 2496]
        "pp16": np.ascontiguousarray(pp16),      # [16, 2608]
        "ycov_row": ycovs.reshape(1, YCN).astype(np.float32),
    }


# ----------------------------------------------------------------------------
# Device kernel (Bass/Tile), SPMD over 8 cores.
# ----------------------------------------------------------------------------

def _build_nc():
    import os
    import concourse.tile as tile
    from concourse import bacc, mybir

    use_f32r = os.environ.get("KF_F32R", "1") == "1"
    use_packy = os.environ.get("KF_PACKY", "1") == "1"
    F32 = mybir.dt.float32
    F32R = mybir.dt.float32r if use_f32r else F32
    nc = bacc.Bacc("TRN2", target_bir_lowering=False, debug=False,
                   num_devices=NCORES)

    # matmul operands are declared float32r in DRAM directly (same bytes as
    # f32 host arrays) so the fast sync/HWDGE DMA path loads them un-cast.
    obs_d = nc.dram_tensor("obs_pack", [P, NCH * G_SH], F32R, kind="ExternalInput").ap()
    pp128_d = nc.dram_tensor("pp128", [P, 2496], F32R, kind="ExternalInput").ap()
    pp16_d = nc.dram_tensor("pp16", [S, 2608], F32R, kind="ExternalInput").ap()
    yc_d = nc.dram_tensor("ycov_row", [1, YCN], F32, kind="ExternalInput").ap()
    ym_d = nc.dram_tensor("ymeant_out", [T * M, G_SH], F32, kind="ExternalOutput").ap()
    cv_d = nc.dram_tensor("covs_out", [G_SH, YCN], F32, kind="ExternalOutput").ap()

    with tile.TileContext(nc) as tc:
        with tc.tile_pool(name="const", bufs=1) as cpool, \
             tc.tile_pool(name="work", bufs=1) as wpool, \
             tc.tile_pool(name="mts", bufs=16) as mpool, \
             tc.tile_pool(name="ypsum", bufs=4, space="PSUM") as ypsum, \
             tc.tile_pool(name="mpsum", bufs=4, space="PSUM") as mpsum:

            # ---- inputs, all on the scalar/ACT HWDGE ring (FIFO per ring:
            # ycov first so the replicate can start ~immediately) ----
            ycv = cpool.tile([1, YCN], F32, tag="ycv")
            nc.scalar.dma_start(out=ycv[:], in_=yc_d)
            pp16_sb = cpool.tile([S, 2608], F32R, tag="pp16")
            nc.scalar.dma_start(out=pp16_sb[:], in_=pp16_d)
            pp128_sb = cpool.tile([P, 2496], F32R, tag="pp128")
            nc.scalar.dma_start(out=pp128_sb[:], in_=pp128_d)
            w_sb = pp128_sb[:, 0:2048]
            bc_sb = pp128_sb[:, 2048:2304]
            bcs_sb = pp128_sb[:, 2304:2496]
            u_sb = pp16_sb[:, 0:2048]
            ac_sb = pp16_sb[:, 2048:2304]
            acs_sb = pp16_sb[:, 2304:2352]
            mt = pp16_sb[:, 2352:2608]

            # replicate ycov [1,16384] across 128 partitions.  First two
            # tiles via DVE stream_shuffle with rows seeded straight from
            # DRAM (no dependency on the ycv SBUF load, so covs writes can
            # start almost immediately); the big tail tiles via gpsimd
            # partition_broadcast.  Widths staggered for early first DMAs.
            off = 0
            for j, rw in enumerate((1024, 3072, 4096, 8192)):
                rep = cpool.tile([P, rw], F32, tag=f"rep{j}", name=f"rep{j}")
                if j < 2:
                    for r in (0, 32, 64, 96):
                        nc.scalar.dma_start(out=rep[r:r + 1, :],
                                            in_=yc_d[0:1, off:off + rw])
                    nc.vector.stream_shuffle(rep[:], rep[:], [0] * 32)
                else:
                    nc.gpsimd.partition_broadcast(
                        rep[:], ycv[0:1, off:off + rw])
                for gh in range(2):
                    nc.sync.dma_start(
                        out=cv_d[gh * P:(gh + 1) * P, off:off + rw],
                        in_=rep[:])
                off += rw

            obs_sb = []
            for j in range(2):
                ob = cpool.tile([P, 8 * G_SH], F32R, tag=f"obs{j}", name=f"obs{j}")
                nc.scalar.dma_start(out=ob[:], in_=obs_d[:, j * 8 * G_SH:(j + 1) * 8 * G_SH])
                obs_sb.append(ob)

            # ---- two-level mean scan ----
            def obs_rhs(k):
                return obs_sb[k // 8][:, (k % 8) * G_SH:(k % 8 + 1) * G_SH]

            # phase A: super-chunk boundary chain mt_0 -> mt_4 -> mt_8 -> mt_12
            mts = [None, None, None, None]
            mts[0] = mt
            for J in range(3):
                pm = mpsum.tile([S, G_SH], F32, tag="pm")
                nc.tensor.matmul(out=pm[:], lhsT=acs_sb[:, J * S:(J + 1) * S],
                                 rhs=mts[J][:], start=True, stop=False)
                for i in range(4):
                    col = (J * 4 + i) * S
                    nc.tensor.matmul(out=pm[:], lhsT=bcs_sb[:, col:col + S],
                                     rhs=obs_rhs(4 * J + i), start=False, stop=(i == 3))
                nxt = mpool.tile([S, G_SH], F32R, tag="mt")
                nc.vector.tensor_copy(out=nxt[:], in_=pm[:])
                mts[J + 1] = nxt

            # phase B+C: four independent inner carry chains, interleaved so
            # the PE always has ready work; y-matmuls emitted as soon as
            # their chunk's mean state exists.  dst AP of the packed means
            # DMA is iterated [p][c][g] to match the SBUF src layout.
            mtk = [None] * NCH
            for J in range(4):
                mtk[4 * J] = mts[J]
            ypack = wpool.tile([P, NCH * G_SH], F32, tag="ypack")
            ydone = [0]

            def emit_y(k):
                py = ypsum.tile([P, G_SH], F32, tag="py")
                nc.tensor.matmul(out=py[:], lhsT=w_sb[:, k * CM:(k + 1) * CM],
                                 rhs=obs_rhs(k), start=True, stop=False)
                nc.tensor.matmul(out=py[:], lhsT=u_sb[:, k * CM:(k + 1) * CM],
                                 rhs=mtk[k][:], start=False, stop=True)
                nc.scalar.copy(out=ypack[:, k * G_SH:(k + 1) * G_SH], in_=py[:])
                ydone[0] += 1
                if ydone[0] == NCH:
                    # one DMA, last in program order -> last sem lane user
                    nc.scalar.dma_start(
                        out=ym_d.rearrange("(k p) g -> p k g", p=P),
                        in_=ypack[:])

            for J in range(4):
                emit_y(4 * J)
            for h in range(1, 4):
                for J in range(4):
                    k = 4 * J + h - 1            # hop producing mt_{k+1}
                    pm = mpsum.tile([S, G_SH], F32, tag="pm")
                    nc.tensor.matmul(out=pm[:], lhsT=ac_sb[:, k * S:(k + 1) * S],
                                     rhs=mtk[k][:], start=True, stop=False)
                    nc.tensor.matmul(out=pm[:], lhsT=bc_sb[:, k * S:(k + 1) * S],
                                     rhs=obs_rhs(k), start=False, stop=True)
                    nxt = mpool.tile([S, G_SH], F32R, tag="mt")
                    nc.vector.tensor_copy(out=nxt[:], in_=pm[:])
                    mtk[k + 1] = nxt
                    emit_y(k + 1)

    nc.compile()
    return nc


def _get_nc():
    if "nc" not in _CACHE:
        _CACHE["nc"] = _build_nc()
    return _CACHE["nc"]


# ----------------------------------------------------------------------------
# Entry point
# ----------------------------------------------------------------------------

def kernel(input, init_state_mean, init_log_diag, init_off_diag, F, H, L_Q, L_R,
           **run_kwargs):
    from concourse.bass_utils import run_bass_kernel_spmd

    params = _compute_params(init_state_mean, init_log_diag, init_off_diag,
                             F, H, L_Q, L_R)

    inp = np.ascontiguousarray(np.asarray(input, np.float32))
    # obsT[(t*M+m), g], then per-core pack: [16, 128, G_SH] -> [128, 16*G_SH]
    obsT = inp.transpose(1, 2, 0).reshape(T * M, G)

    in_maps = []
    for i in range(NCORES):
        shard = obsT[:, i * G_SH:(i + 1) * G_SH]
        obs_pack = np.ascontiguousarray(
            shard.reshape(NCH, P, G_SH).transpose(1, 0, 2).reshape(P, NCH * G_SH))
        m = dict(params)
        m["obs_pack"] = obs_pack
        in_maps.append(m)

    nc = _get_nc()
    res = run_bass_kernel_spmd(nc, in_maps, core_ids=list(range(NCORES)),
                               **run_kwargs)

    means = np.empty((G, T, M), np.float32)
    covs = np.empty((G, T, M, M), np.float32)
    for i in range(NCORES):
        out = res.results[i]
        means[i * G_SH:(i + 1) * G_SH] = out["ymeant_out"].T.reshape(G_SH, T, M)
        covs[i * G_SH:(i + 1) * G_SH] = out["covs_out"].reshape(G_SH, T, M, M)
    if run_kwargs:
        _CACHE["last_results"] = res
    return means, covs


# revision 17
# speedup vs baseline: 1.3140x; 1.3140x over previous
"""Trainium2 Bass kernel for the batched Kalman filter problem.

Problem: G=2048 groups, T=256 steps, M=8 obs dims, S=16 state dims.
Output: means [G,T,M], covs [G,T,M,M] (the per-step predicted measurement
distribution).

Structure exploited:
  * The covariance recursion P_t is data-independent and P0 is broadcast, so
    P_t / the Kalman gain / the output covariance ycov_t are identical for
    every group.  ycov [T,M,M] is computed on host (tiny, sequential 16x16
    math) and the device broadcasts it into the [G,T,M,M] output — that
    broadcast write (134 MB) is the memory-bound bulk of the work.
  * The mean recursion is affine with shared coefficient matrices:
        mean_{t+1} = mean_t @ A_t + obs_t @ B_t,   ymean_t = mean_t @ H^T
    Chunking t into blocks of C=16 turns this into a short sequential chain
    of dense matmuls over the whole group batch (device, TensorE).

Sharding: groups split across 8 cores (256 groups/core), pure data parallel.

Device per-core layout (all f32):
  obs_pack  [128, 16*256] in  : col block k = obsT rows [128k:128k+128)
                                (obsT[(t*M+m), g] = input[g, t, m])
  w_pack    [128, 16*128] in  : col block k = W_k   (lhsT, K=(j,m') x N=(i,m))
  u_pack    [16,  16*128] in  : col block k = U_k   (lhsT, K=s x N=(i,m))
  acar_pack [16,  16*16]  in  : col block k = Acar_k (lhsT)
  bcar_pack [128, 16*16]  in  : col block k = Bcar_k (lhsT)
  mean0_t   [16, 256]     in  : initial mean broadcast over groups (meanT)
  ycov_row  [1, 16384]    in  : ycovs [T*M*M] flattened
  ymeant_out [2048, 256]  out : ymeanT[(t*M+m), g]  (host transposes back)
  covs_out   [256, 16384] out : per-group covs rows (identical), g-major
"""

import numpy as np

G, T, M, S = 2048, 256, 8, 16
NCORES = 8
G_SH = G // NCORES            # 256 groups per core
C = 16                        # time chunk
NCH = T // C                  # 16 chunks
CM = C * M                    # 128 = rows/cols of W per chunk
P = 128
YCN = T * M * M               # 16384 floats of ycov
REP_TILES = 4                 # split replicated ycov into 4 [128, 4096] tiles
REP_W = YCN // REP_TILES      # 4096

_CACHE: dict = {}


def _install_ntff_hook():
    """Provide antenv.axon_hooks (absent in this image) so bass_utils can
    NTFF-profile under axon when trace=True.  No-op if already present."""
    import sys
    import types
    import contextlib

    try:
        import antenv.axon_hooks  # noqa: F401
        return
    except ImportError:
        pass

    mod = types.ModuleType("antenv.axon_hooks")
    state = {"hook": None}
    mod.set_axon_ntff_profile_hook = lambda h: state.__setitem__("hook", h)
    mod.get_axon_ntff_profile_hook = lambda: state["hook"]
    sys.modules["antenv.axon_hooks"] = mod
    try:
        import antenv
        antenv.axon_hooks = mod
    except ImportError:
        pass

    so_path = "/opt/axon/libaxon_pjrt.so"
    try:
        import ctypes
        lib = ctypes.CDLL(so_path)
        if not hasattr(lib, "axon_start_nrt_profile"):
            return
        lib.axon_start_nrt_profile.argtypes = [
            ctypes.POINTER(ctypes.c_int64), ctypes.c_size_t]
        lib.axon_start_nrt_profile.restype = ctypes.c_int64
        lib.axon_stop_nrt_profile.argtypes = [ctypes.c_char_p]
        lib.axon_stop_nrt_profile.restype = ctypes.c_int64

        @contextlib.contextmanager
        def _hook(output_dir, device_ids):
            import jax
            jax.devices()
            if device_ids:
                ids = (ctypes.c_int64 * len(device_ids))(*device_ids)
                rc = lib.axon_start_nrt_profile(ids, len(device_ids))
            else:
                rc = lib.axon_start_nrt_profile(None, 0)
            if rc != 0:
                raise RuntimeError(f"axon_start_nrt_profile rc={rc}")
            try:
                yield
            finally:
                n = lib.axon_stop_nrt_profile(str(output_dir).encode())
                import sys as _s
                print(f"ntff profile: {n} file(s) -> {output_dir}", file=_s.stderr)

        state["hook"] = _hook
    except OSError:
        pass


_install_ntff_hook()


# ----------------------------------------------------------------------------
# Host-side math: tiny data-independent Kalman recursions (float64).
# ----------------------------------------------------------------------------

def _compute_params(init_state_mean, init_log_diag, init_off_diag, F, H, L_Q, L_R):
    F = np.asarray(F, np.float64)
    H = np.asarray(H, np.float64)
    L_Q = np.asarray(L_Q, np.float64)
    L_R = np.asarray(L_R, np.float64)
    Q = L_Q @ L_Q.T + 1e-4 * np.eye(S)
    R = L_R @ L_R.T + 1e-4 * np.eye(M)
    L0 = np.zeros((S, S))
    r, c = np.tril_indices(S, -1)
    L0[r, c] = np.asarray(init_off_diag, np.float64)
    L0 += np.diag(np.exp(np.asarray(init_log_diag, np.float64)))
    Pm = L0 @ L0.T

    I_S = np.eye(S)
    ycovs = np.zeros((T, M, M))
    A = np.zeros((T - 1, S, S))
    B = np.zeros((T - 1, M, S))
    ycovs[0] = H @ Pm @ H.T + R
    for j in range(T - 1):
        HP = H @ Pm
        Smat = HP @ H.T + R
        Kt = np.linalg.solve(Smat, HP)          # [M,S] = K^T
        P_u = Pm - Kt.T @ HP
        A[j] = (I_S - H.T @ Kt) @ F.T
        B[j] = Kt @ F.T
        Pm = F @ P_u @ F.T + Q
        ycovs[j + 1] = H @ Pm @ H.T + R

    Ht = H.T

    def getA(j):
        return A[j] if j < T - 1 else I_S

    def getB(j):
        return B[j] if j < T - 1 else np.zeros((M, S))

    U = np.zeros((NCH, S, CM))
    W = np.zeros((NCH, CM, CM))
    Acar = np.zeros((NCH, S, S))
    Bcar = np.zeros((NCH, CM, S))
    for k in range(NCH):
        t0 = k * C
        pref = I_S.copy()
        for i in range(C):
            U[k, :, i * M:(i + 1) * M] = pref @ Ht
            pref = pref @ getA(t0 + i)
        Acar[k] = pref
        for j in range(C):
            Bj = getB(t0 + j)
            mid = I_S.copy()
            for i in range(j + 1, C):
                W[k, j * M:(j + 1) * M, i * M:(i + 1) * M] = Bj @ mid @ Ht
                mid = mid @ getA(t0 + i)
            Bcar[k, j * M:(j + 1) * M, :] = Bj @ mid

    # super-chunk (4 chunks = 64 steps) transition composites for the
    # two-level scan: mt_{4(J+1)} = AcarS_J^T mt_{4J} + sum_i BcarS_J[i]^T obs_{4J+i}
    NSUP = NCH // 4 - 1                          # 3 boundary hops
    AcarS = np.zeros((NSUP, S, S))
    BcarS = np.zeros((NSUP, 4, CM, S))           # per obs chunk 4J+i
    for J in range(NSUP):
        pr = I_S.copy()
        for i in range(3, -1, -1):               # suffix products of Acar
            BcarS[J, i] = Bcar[4 * J + i] @ pr
            pr = Acar[4 * J + i] @ pr
        AcarS[J] = pr

    def pack(x):  # [NCH, p, n] -> [p, NCH*n], col block k = x[k]
        return np.ascontiguousarray(
            x.transpose(1, 0, 2).reshape(x.shape[1], -1)).astype(np.float32)

    mean0 = np.asarray(init_state_mean, np.float64)
    # two packed param tensors (one DMA each) to keep total DMA count low —
    # Tile has only 8 DMA-completion sem lanes and lane reuse creates false
    # cross-DMA ordering when late DMAs precede early ones in lane order.
    pp128 = np.concatenate([
        pack(W),                                           # [:, 0:2048]
        pack(Bcar),                                        # [:, 2048:2304]
        np.ascontiguousarray(                              # [:, 2304:2496]
            BcarS.transpose(2, 0, 1, 3).reshape(CM, -1)).astype(np.float32),
    ], axis=1)
    pp16 = np.concatenate([
        pack(U),                                           # [:, 0:2048]
        pack(Acar),                                        # [:, 2048:2304]
        pack(AcarS),                                       # [:, 2304:2352]
        np.ascontiguousarray(                              # [:, 2352:2608]
            np.broadcast_to(mean0[:, None], (S, G_SH))).astype(np.float32),
    ], axis=1)
    return {
        "pp128": np.ascontiguousarray(pp128),    # [128, 2496]
        "pp16": np.ascontiguousarray(pp16),      # [16, 2608]
        "ycov_row": ycovs.reshape(1, YCN).astype(np.float32),
    }


# ----------------------------------------------------------------------------
# Device kernel (Bass/Tile), SPMD over 8 cores.
# ----------------------------------------------------------------------------

def _build_nc():
    import os
    import concourse.tile as tile
    from concourse import bacc, mybir

    use_f32r = os.environ.get("KF_F32R", "1") == "1"
    use_packy = os.environ.get("KF_PACKY", "1") == "1"
    F32 = mybir.dt.float32
    F32R = mybir.dt.float32r if use_f32r else F32
    nc = bacc.Bacc("TRN2", target_bir_lowering=False, debug=False,
                   num_devices=NCORES)

    # matmul operands are declared float32r in DRAM directly (same bytes as
    # f32 host arrays) so the fast sync/HWDGE DMA path loads them un-cast.
    obs_d = nc.dram_tensor("obs_pack", [P, NCH * G_SH], F32R, kind="ExternalInput").ap()
    pp128_d = nc.dram_tensor("pp128", [P, 2496], F32R, kind="ExternalInput").ap()
    pp16_d = nc.dram_tensor("pp16", [S, 2608], F32R, kind="ExternalInput").ap()
    yc_d = nc.dram_tensor("ycov_row", [1, YCN], F32, kind="ExternalInput").ap()
    ym_d = nc.dram_tensor("ymeant_out", [T * M, G_SH], F32, kind="ExternalOutput").ap()
    cv_d = nc.dram_tensor("covs_out", [G_SH, YCN], F32, kind="ExternalOutput").ap()

    with tile.TileContext(nc) as tc:
        with tc.tile_pool(name="const", bufs=1) as cpool, \
             tc.tile_pool(name="work", bufs=1) as wpool, \
             tc.tile_pool(name="mts", bufs=16) as mpool, \
             tc.tile_pool(name="ypsum", bufs=4, space="PSUM") as ypsum, \
             tc.tile_pool(name="mpsum", bufs=4, space="PSUM") as mpsum:

            # ---- inputs, all on the scalar/ACT HWDGE ring (FIFO per ring:
            # ycov first so the replicate can start ~immediately) ----
            ycv = cpool.tile([1, YCN], F32, tag="ycv")
            nc.scalar.dma_start(out=ycv[:], in_=yc_d)
            pp16_sb = cpool.tile([S, 2608], F32R, tag="pp16")
            nc.scalar.dma_start(out=pp16_sb[:], in_=pp16_d)
            pp128_sb = cpool.tile([P, 2496], F32R, tag="pp128")
            nc.scalar.dma_start(out=pp128_sb[:], in_=pp128_d)
            w_sb = pp128_sb[:, 0:2048]
            bc_sb = pp128_sb[:, 2048:2304]
            bcs_sb = pp128_sb[:, 2304:2496]
            u_sb = pp16_sb[:, 0:2048]
            ac_sb = pp16_sb[:, 2048:2304]
            acs_sb = pp16_sb[:, 2304:2352]
            mt = pp16_sb[:, 2352:2608]

            obs_sb = []
            for j in range(2):
                ob = cpool.tile([P, 8 * G_SH], F32R, tag=f"obs{j}", name=f"obs{j}")
                nc.scalar.dma_start(out=ob[:], in_=obs_d[:, j * 8 * G_SH:(j + 1) * 8 * G_SH])
                obs_sb.append(ob)

            # replicate ycov [1,16384] across 128 partitions on gpsimd (idle
            # engine; Vector stays free for the scan's carry-chain casts).
            # Widths staggered so the first covs DMAs start early; covs
            # writes are the only sync-queue DMAs (no head-of-line blocking).
            off = 0
            for j, rw in enumerate((1024, 3072, 4096, 8192)):
                rep = cpool.tile([P, rw], F32, tag=f"rep{j}", name=f"rep{j}")
                nc.gpsimd.partition_broadcast(
                    rep[:], ycv[0:1, off:off + rw])
                for gh in range(2):
                    nc.sync.dma_start(
                        out=cv_d[gh * P:(gh + 1) * P, off:off + rw],
                        in_=rep[:])
                off += rw

            # ---- two-level mean scan ----
            def obs_rhs(k):
                return obs_sb[k // 8][:, (k % 8) * G_SH:(k % 8 + 1) * G_SH]

            # phase A: super-chunk boundary chain mt_0 -> mt_4 -> mt_8 -> mt_12
            mts = [None, None, None, None]
            mts[0] = mt
            for J in range(3):
                pm = mpsum.tile([S, G_SH], F32, tag="pm")
                nc.tensor.matmul(out=pm[:], lhsT=acs_sb[:, J * S:(J + 1) * S],
                                 rhs=mts[J][:], start=True, stop=False)
                for i in range(4):
                    col = (J * 4 + i) * S
                    nc.tensor.matmul(out=pm[:], lhsT=bcs_sb[:, col:col + S],
                                     rhs=obs_rhs(4 * J + i), start=False, stop=(i == 3))
                nxt = mpool.tile([S, G_SH], F32R, tag="mt")
                nc.vector.tensor_copy(out=nxt[:], in_=pm[:])
                mts[J + 1] = nxt

            # phase B+C: four independent inner carry chains, interleaved so
            # the PE always has ready work; y-matmuls emitted as soon as
            # their chunk's mean state exists.  dst AP of the packed means
            # DMA is iterated [p][c][g] to match the SBUF src layout.
            mtk = [None] * NCH
            for J in range(4):
                mtk[4 * J] = mts[J]
            ypack = wpool.tile([P, NCH * G_SH], F32, tag="ypack")
            ydone = [0]

            def emit_y(k):
                py = ypsum.tile([P, G_SH], F32, tag="py")
                nc.tensor.matmul(out=py[:], lhsT=w_sb[:, k * CM:(k + 1) * CM],
                                 rhs=obs_rhs(k), start=True, stop=False)
                nc.tensor.matmul(out=py[:], lhsT=u_sb[:, k * CM:(k + 1) * CM],
                                 rhs=mtk[k][:], start=False, stop=True)
                nc.scalar.copy(out=ypack[:, k * G_SH:(k + 1) * G_SH], in_=py[:])
                ydone[0] += 1
                if ydone[0] == NCH:
                    # one DMA, last in program order -> last sem lane user
                    nc.scalar.dma_start(
                        out=ym_d.rearrange("(k p) g -> p k g", p=P),
                        in_=ypack[:])

            for J in range(4):
                emit_y(4 * J)
            for h in range(1, 4):
                for J in range(4):
                    k = 4 * J + h - 1            # hop producing mt_{k+1}
                    pm = mpsum.tile([S, G_SH], F32, tag="pm")
                    nc.tensor.matmul(out=pm[:], lhsT=ac_sb[:, k * S:(k + 1) * S],
                                     rhs=mtk[k][:], start=True, stop=False)
                    nc.tensor.matmul(out=pm[:], lhsT=bc_sb[:, k * S:(k + 1) * S],
                                     rhs=obs_rhs(k), start=False, stop=True)
                    nxt = mpool.tile([S, G_SH], F32R, tag="mt")
                    nc.vector.tensor_copy(out=nxt[:], in_=pm[:])
                    mtk[k + 1] = nxt
                    emit_y(k + 1)

    nc.compile()
    return nc


def _get_nc():
    if "nc" not in _CACHE:
        _CACHE["nc"] = _build_nc()
    return _CACHE["nc"]


# ----------------------------------------------------------------------------
# Entry point
# ----------------------------------------------------------------------------

def kernel(input, init_state_mean, init_log_diag, init_off_diag, F, H, L_Q, L_R,
           **run_kwargs):
    from concourse.bass_utils import run_bass_kernel_spmd

    params = _compute_params(init_state_mean, init_log_diag, init_off_diag,
                             F, H, L_Q, L_R)

    inp = np.ascontiguousarray(np.asarray(input, np.float32))
    # obsT[(t*M+m), g], then per-core pack: [16, 128, G_SH] -> [128, 16*G_SH]
    obsT = inp.transpose(1, 2, 0).reshape(T * M, G)

    in_maps = []
    for i in range(NCORES):
        shard = obsT[:, i * G_SH:(i + 1) * G_SH]
        obs_pack = np.ascontiguousarray(
            shard.reshape(NCH, P, G_SH).transpose(1, 0, 2).reshape(P, NCH * G_SH))
        m = dict(params)
        m["obs_pack"] = obs_pack
        in_maps.append(m)

    nc = _get_nc()
    res = run_bass_kernel_spmd(nc, in_maps, core_ids=list(range(NCORES)),
                               **run_kwargs)

    means = np.empty((G, T, M), np.float32)
    covs = np.empty((G, T, M, M), np.float32)
    for i in range(NCORES):
        out = res.results[i]
        means[i * G_SH:(i + 1) * G_SH] = out["ymeant_out"].T.reshape(G_SH, T, M)
        covs[i * G_SH:(i + 1) * G_SH] = out["covs_out"].reshape(G_SH, T, M, M)
    if run_kwargs:
        _CACHE["last_results"] = res
    return means, covs
